# revision 7
# baseline (speedup 1.0000x reference)
"""Causal multi-head attention (RMSNorm + QKV + causal softmax + out-proj)
for Trainium2, sharded over 8 NeuronCores.

Sharding: data-parallel over batch (2) x tensor-parallel over head-groups
(16 heads -> 4 groups of 4). Core c = 4*b + hg computes
    partial_out[b] = Attn_heads[4hg:4hg+4](x[b]) @ Wo[256hg:256hg+256, :]
and the host sums the 4 head-group partials per batch (the TP unshard).

On-device dataflow (per core, all matmuls in float32r = full PE rate):
  x (f32) -> RMSNorm stats (ACT square+accum) -> xn = x*s (DVE) ->
  PE-transpose -> xnT [dim, seq] -> Q^T/K^T (d-major) and V [seq, d] projs ->
  per head-pair: S^T = K^T.T Q^T (two K=64 matmuls packed via tile_position),
  P^T = exp(S^T/8) (ACT, no max-subtraction: |scores| <= ~3.3),
  causal via block skipping + one triangular mask multiply on diagonal
  128-col windows, O^T = Vaug.T P^T with a ones-column giving row sums,
  normalize by DVE reciprocal + outer-product broadcast (PE) + multiply,
  out = A @ Wo accumulated over the two 128-row halves of Wo.
"""
import os
import sys
import functools

for _p in ("/opt/trn_rl_repo", os.path.expanduser("~/.axon_site/_ro/trn_rl_repo")):
    if os.path.isdir(_p) and _p not in sys.path:
        sys.path.insert(0, _p)

import numpy as np

B = 2
N = 2048
DIM = 1024
HEADS = 16
DH = 64
SCALE = DH ** -0.5   # 0.125
NCORES = 8
NGROUPS = 4          # head groups (tensor parallel)
HPC = HEADS // NGROUPS  # 4 heads per core
P = 128
RC = 4               # row chunks of 512 for projections / q-chunks
QCHUNK = 512
NKB = N // P         # 16 key blocks


def _build():
    import concourse.bass as bass
    import concourse.mybir as mybir
    import concourse.tile as tile
    from concourse import bacc

    dt = mybir.dt
    f32 = dt.float32
    f32r = dt.float32r
    bf16 = dt.bfloat16
    AF = mybir.ActivationFunctionType
    ALU = mybir.AluOpType

    nc = bacc.Bacc("TRN2", target_bir_lowering=False, debug=False,
                   num_devices=NCORES)

    x_d = nc.dram_tensor("x", [N, DIM], f32, kind="ExternalInput")
    wq_d = nc.dram_tensor("wq", [DIM, HPC * DH], f32, kind="ExternalInput")
    wk_d = nc.dram_tensor("wk", [DIM, HPC * DH], f32, kind="ExternalInput")
    wv_d = nc.dram_tensor("wv", [DIM, HPC * DH], f32, kind="ExternalInput")
    wo_d = nc.dram_tensor("wo", [HPC * DH, DIM], f32, kind="ExternalInput")
    gm_d = nc.dram_tensor("gammat", [P, 8], f32, kind="ExternalInput")
    mb_d = nc.dram_tensor("maskbias", [P, NKB], f32, kind="ExternalInput")
    tri_d = nc.dram_tensor("tri", [P, P], f32, kind="ExternalInput")
    id_d = nc.dram_tensor("ident", [P, P], f32, kind="ExternalInput")
    on_d = nc.dram_tensor("onesin", [1, DH], f32, kind="ExternalInput")
    vo_d = nc.dram_tensor("vones", [P, NKB * HPC], f32, kind="ExternalInput")
    out_d = nc.dram_tensor("out", [N, DIM], f32, kind="ExternalOutput")

    with tile.TileContext(nc) as tc:
        with (
            tc.tile_pool(name="consts", bufs=1) as consts,
            tc.tile_pool(name="wpool", bufs=1) as wpool,
            tc.tile_pool(name="big", bufs=1) as big,
        ):
            # ---- constant / weight loads
            ident = consts.tile([P, P], f32r)
            nc.gpsimd.dma_start(ident[:], id_d[:])
            tri = consts.tile([P, P], f32r)
            nc.gpsimd.dma_start(tri[:], tri_d[:])
            gammat = consts.tile([P, 8], f32)
            nc.sync.dma_start(gammat[:], gm_d[:])
            maskb = consts.tile([P, NKB], f32)
            nc.sync.dma_start(maskb[:], mb_d[:])
            onesr = consts.tile([1, DH], f32r)
            nc.gpsimd.dma_start(onesr[:], on_d[:])

            wq = wpool.tile([P, 8, HPC * DH], f32r)
            wk = wpool.tile([P, 8, HPC * DH], f32r)
            wv = wpool.tile([P, 8, HPC * DH], f32r)
            nc.gpsimd.dma_start(wq[:], wq_d.ap().rearrange("(k p) c -> p k c", p=P))
            nc.gpsimd.dma_start(wk[:], wk_d.ap().rearrange("(k p) c -> p k c", p=P))
            nc.gpsimd.dma_start(wv[:], wv_d.ap().rearrange("(k p) c -> p k c", p=P))
            wo = wpool.tile([P, 2, DIM], f32r)
            nc.gpsimd.dma_start(wo[:], wo_d.ap().rearrange("(hp p) c -> p hp c", p=P))
            # fold gamma into W rows: W[f, :] *= gamma[f]
            for k in range(8):
                for w in (wq, wk, wv):
                    nc.vector.tensor_scalar(
                        out=w[:, k, :], in0=w[:, k, :],
                        scalar1=gammat[:, k:k + 1], scalar2=None, op0=ALU.mult)

            # ---- persistent activations
            qt = big.tile([P, 2, N], f32r)     # Q^T: [d-of-pair, hp, seq]
            kt = big.tile([P, 2, N], f32r)
            vt = big.tile([P, NKB, HPC, DH + 1], f32r)   # V rows + ones col
            nc.gpsimd.dma_start(
                vt[:, :, :, DH:DH + 1],
                vo_d.ap().rearrange("p (kb h) -> p kb h", h=HPC).unsqueeze(3))
            a0 = big.tile([P, N], f32r)        # A^T for head pair 0
            a1 = big.tile([P, N], f32r)
            ss = big.tile([P, 16], f32)        # row sum-of-squares
            sfac = big.tile([P, 16], f32)      # 32 / max(sqrt(ss), 1e-12)

            # ================= Phase 1: norm, transpose, projections ======
            with (
                tc.tile_pool(name="xin", bufs=6) as xin,
                tc.tile_pool(name="sq", bufs=2) as sqp,
                tc.tile_pool(name="xn", bufs=3) as xnp,
                tc.tile_pool(name="xnt", bufs=2) as xntp,
                tc.tile_pool(name="trps", bufs=2, space="PSUM") as trps,
                tc.tile_pool(name="pjps", bufs=3, space="PSUM") as pjps,
            ):
                for rc in range(RC):
                    xtiles = []
                    for t in range(4):
                        ti = rc * 4 + t
                        xt = xin.tile([P, DIM], f32, tag="x")
                        nc.sync.dma_start(xt[:], x_d[ti * P:(ti + 1) * P, :])
                        xtiles.append(xt)
                        scr = sqp.tile([P, DIM], bf16, tag="sq")
                        nc.scalar.activation(scr[:], xt[:], AF.Square,
                                             accum_out=ss[:, ti:ti + 1])
                    # stats for this row chunk ([128, 4] batch)
                    sl = slice(rc * 4, rc * 4 + 4)
                    nc.scalar.activation(sfac[:, sl], ss[:, sl], AF.Sqrt)
                    nc.vector.tensor_scalar(out=sfac[:, sl], in0=sfac[:, sl],
                                            scalar1=1e-12, scalar2=None,
                                            op0=ALU.max)
                    nc.vector.reciprocal(sfac[:, sl], sfac[:, sl])
                    nc.vector.tensor_scalar(out=sfac[:, sl], in0=sfac[:, sl],
                                            scalar1=float(DIM ** 0.5), scalar2=None,
                                            op0=ALU.mult)

                    xnt = xntp.tile([P, 8, QCHUNK], f32r, tag="xnt")
                    for t in range(4):
                        ti = rc * 4 + t
                        xn = xnp.tile([P, DIM], f32r, tag="xn")
                        nc.vector.tensor_scalar(out=xn[:], in0=xtiles[t][:],
                                                scalar1=sfac[:, ti:ti + 1],
                                                scalar2=None, op0=ALU.mult)
                        for kg in range(2):
                            tp = trps.tile([P, 4, P], f32r, tag="tr")
                            for k4 in range(4):
                                k = kg * 4 + k4
                                nc.tensor.transpose(tp[:, k4, :],
                                                    xn[:, k * P:(k + 1) * P],
                                                    ident[:])
                            nc.vector.tensor_copy(
                                xnt[:, kg * 4:(kg + 1) * 4, t * P:(t + 1) * P],
                                tp[:])

                    # Q/K projections for this row chunk
                    for w, dst in ((wq, qt), (wk, kt)):
                        for cc in range(2):
                            ps = pjps.tile([P, QCHUNK], f32, tag="pj")
                            for k in range(8):
                                nc.tensor.matmul(
                                    ps[:], w[:, k, cc * P:(cc + 1) * P],
                                    xnt[:, k, :],
                                    start=(k == 0), stop=(k == 7))
                            nc.vector.tensor_copy(
                                dst[:, cc, rc * QCHUNK:(rc + 1) * QCHUNK], ps[:])
                    # V projection for the 4 key blocks of this row chunk
                    for t in range(4):
                        kb = rc * 4 + t
                        ps = pjps.tile([P, HPC * DH], f32, tag="pj")
                        for k in range(8):
                            nc.tensor.matmul(
                                ps[:], xnt[:, k, t * P:(t + 1) * P],
                                wv[:, k, :],
                                start=(k == 0), stop=(k == 7))
                        nc.vector.tensor_copy(
                            vt[:, kb, :, 0:DH], ps[:].rearrange("p (h d) -> p h d", d=DH))

            # ================= Phase 2: attention =========================
            with (
                tc.tile_pool(name="sps", bufs=2, space="PSUM") as sps,
                tc.tile_pool(name="ops", bufs=2, space="PSUM") as ops,
                tc.tile_pool(name="pt", bufs=3) as ptp,
                tc.tile_pool(name="nrm", bufs=2) as nrm,
                tc.tile_pool(name="ash", bufs=2) as ashp,
            ):
                for hp, adst in ((0, a0), (1, a1)):
                    ash = ashp.tile([DH, N], f32r, tag="ash")
                    for qc in range(RC):
                        qs = slice(qc * QCHUNK, (qc + 1) * QCHUNK)
                        ot = ops.tile([DH + 1, 2, QCHUNK], f32, tag="o")
                        nkb = 4 * qc + 4
                        for kb in range(nkb):
                            ks = slice(kb * P, (kb + 1) * P)
                            st = sps.tile([P, 2, QCHUNK], f32, tag="s")
                            for h in range(2):
                                nc.tensor.matmul(
                                    st[:, h, :],
                                    kt[h * DH:(h + 1) * DH, hp, ks],
                                    qt[h * DH:(h + 1) * DH, hp, qs],
                                    start=True, stop=True,
                                    tile_position=(h * DH, 0))
                            o = max(0, kb * P - qc * QCHUNK)
                            pt = ptp.tile([P, 2, QCHUNK], f32r, tag="pt")
                            nc.scalar.activation(pt[:, :, o:], st[:, :, o:],
                                                 AF.Exp, scale=SCALE,
                                                 bias=maskb[:, kb:kb + 1])
                            if kb >= 4 * qc:  # diagonal block: triangular mask
                                nc.vector.tensor_tensor(
                                    pt[:, :, o:o + P], pt[:, :, o:o + P],
                                    tri[:, None, :].broadcast_to([P, 2, P]),
                                    ALU.mult)
                            for h in range(2):
                                nc.tensor.matmul(
                                    ot[:, h, o:], vt[:, kb, 2 * hp + h, :],
                                    pt[:, h, o:],
                                    start=(kb == 0), stop=(kb == nkb - 1),
                                    skip_group_check=True)
                        # normalize: A = O[0:64] * (1 / O[64])
                        sums = nrm.tile([1, 2, QCHUNK], f32, tag="sums")
                        nc.vector.tensor_copy(sums[:], ot[DH:DH + 1, :, :])
                        rec = nrm.tile([1, 2, QCHUNK], f32r, tag="rec")
                        with nc.allow_low_precision(reason="f32r softmax recip (1.6e-4)"):
                            nc.vector.reciprocal(rec[:], sums[:])
                        bt = sps.tile([DH, 2, QCHUNK], f32, tag="s")
                        for h in range(2):
                            nc.tensor.matmul(bt[:, h, :], onesr[:],
                                             rec[0:1, h, :],
                                             start=True, stop=True)
                        osb = nrm.tile([DH, 2, QCHUNK], f32, tag="osb")
                        nc.scalar.activation(osb[:], ot[0:DH, :, :], AF.Copy)
                        nc.vector.tensor_tensor(adst[0:DH, qs], osb[:, 0, :],
                                                bt[:, 0, :], ALU.mult)
                        nc.vector.tensor_tensor(ash[:, qs], osb[:, 1, :],
                                                bt[:, 1, :], ALU.mult)
                    # move head h'=1 rows into partitions 64..127 of A
                    nc.sync.dma_start(adst[DH:2 * DH, :], ash[:])

            # ================= Phase 3: output projection =================
            with (
                tc.tile_pool(name="outp", bufs=2) as outp,
                tc.tile_pool(name="oj", bufs=3, space="PSUM") as ojps,
            ):
                for r in range(N // P):
                    rs = slice(r * P, (r + 1) * P)
                    orow = outp.tile([P, DIM], f32, tag="orow")
                    for cc in range(2):
                        ps = ojps.tile([P, QCHUNK], f32, tag="oj")
                        for hp, a in ((0, a0), (1, a1)):
                            nc.tensor.matmul(
                                ps[:], a[:, rs], wo[:, hp, cc * QCHUNK:(cc + 1) * QCHUNK],
                                start=(hp == 0), stop=(hp == 1))
                        nc.scalar.activation(orow[:, cc * QCHUNK:(cc + 1) * QCHUNK],
                                             ps[:], AF.Copy)
                    nc.sync.dma_start(out_d[rs, :], orow[:])

    nc.compile()
    return nc


_CACHE = {}


def _get_nc():
    if "nc" not in _CACHE:
        _CACHE["nc"] = _build()
    return _CACHE["nc"]


def kernel(x, mask, gamma, Wq, Wkv, Wo):
    from concourse import bass_utils

    x = np.ascontiguousarray(np.asarray(x, dtype=np.float32))
    mask = np.asarray(mask)
    gamma = np.asarray(gamma, dtype=np.float32)
    Wq = np.asarray(Wq, dtype=np.float32)
    Wkv = np.asarray(Wkv, dtype=np.float32)
    Wo = np.asarray(Wo, dtype=np.float32)

    gammat = np.ascontiguousarray(gamma.reshape(8, P).T)
    tri = (np.arange(P)[None, :] >= np.arange(P)[:, None]).astype(np.float32)
    ident = np.eye(P, dtype=np.float32)

    in_maps = []
    for c in range(NCORES):
        b, hg = divmod(c, NGROUPS)
        cs = slice(hg * HPC * DH, (hg + 1) * HPC * DH)
        mb = np.where(mask[b], 0.0, -1e30).astype(np.float32)
        in_maps.append({
            "x": x[b],
            "wq": np.ascontiguousarray(Wq[:, cs]),
            "wk": np.ascontiguousarray(Wkv[:, :DIM][:, cs]),
            "wv": np.ascontiguousarray(Wkv[:, DIM:][:, cs]),
            "wo": np.ascontiguousarray(Wo[cs, :]),
            "gammat": gammat,
            "maskbias": np.ascontiguousarray(mb.reshape(NKB, P).T),
            "tri": tri,
            "ident": ident,
            "onesin": np.ones((1, DH), dtype=np.float32),
            "vones": np.ones((P, NKB * HPC), dtype=np.float32),
        })

    nc = _get_nc()
    _CACHE["last_in_maps"] = in_maps
    res = bass_utils.run_bass_kernel_spmd(nc, in_maps, core_ids=list(range(NCORES)))
    out = np.zeros((B, N, DIM), dtype=np.float32)
    for c in range(NCORES):
        b = c // NGROUPS
        out[b] += res.results[c]["out"]
    return out


# revision 11
# speedup vs baseline: 1.2080x; 1.2080x over previous
"""Causal multi-head attention (RMSNorm + QKV + causal softmax + out-proj)
for Trainium2, sharded over 8 NeuronCores.

Sharding: data-parallel over batch (2) x tensor-parallel over head-groups
(16 heads -> 4 groups of 4). Core c = 4*b + hg computes
    partial_out[b] = Attn_heads[4hg:4hg+4](x[b]) @ Wo[256hg:256hg+256, :]
and the host sums the 4 head-group partials per batch (the TP unshard).

On-device dataflow (per core, all matmuls in float32r = full PE rate):
  x (f32) -> RMSNorm stats (ACT square+accum) -> xn = x*s (DVE) ->
  PE-transpose -> xnT [dim, seq] -> Q^T/K^T (d-major) and V [seq, d] projs ->
  per head-pair: S^T = K^T.T Q^T (two K=64 matmuls packed via tile_position),
  P^T = exp(S^T/8) (ACT, no max-subtraction: |scores| <= ~3.3),
  causal via block skipping + one triangular mask multiply on diagonal
  128-col windows, O^T = Vaug.T P^T with a ones-column giving row sums,
  normalize by DVE reciprocal + outer-product broadcast (PE) + multiply,
  out = A @ Wo accumulated over the two 128-row halves of Wo.
"""
import os
import sys
import functools

for _p in ("/opt/trn_rl_repo", os.path.expanduser("~/.axon_site/_ro/trn_rl_repo")):
    if os.path.isdir(_p) and _p not in sys.path:
        sys.path.insert(0, _p)

import numpy as np

B = 2
N = 2048
DIM = 1024
HEADS = 16
DH = 64
SCALE = DH ** -0.5   # 0.125
NCORES = 8
NGROUPS = 4          # head groups (tensor parallel)
HPC = HEADS // NGROUPS  # 4 heads per core
P = 128
RC = 4               # row chunks of 512 for projections / q-chunks
QCHUNK = 512
NKB = N // P         # 16 key blocks


def _build():
    import concourse.bass as bass
    import concourse.mybir as mybir
    import concourse.tile as tile
    from concourse import bacc

    dt = mybir.dt
    f32 = dt.float32
    f32r = dt.float32r
    bf16 = dt.bfloat16
    AF = mybir.ActivationFunctionType
    ALU = mybir.AluOpType

    nc = bacc.Bacc("TRN2", target_bir_lowering=False, debug=False,
                   num_devices=NCORES)

    x_d = nc.dram_tensor("x", [N, DIM], f32, kind="ExternalInput")
    wq_d = nc.dram_tensor("wq", [DIM, HPC * DH], f32, kind="ExternalInput")
    wk_d = nc.dram_tensor("wk", [DIM, HPC * DH], f32, kind="ExternalInput")
    wv_d = nc.dram_tensor("wv", [DIM, HPC * DH], f32, kind="ExternalInput")
    wo_d = nc.dram_tensor("wo", [HPC * DH, DIM], f32, kind="ExternalInput")
    gm_d = nc.dram_tensor("gammat", [P, 8], f32, kind="ExternalInput")
    mb_d = nc.dram_tensor("maskbias", [P, NKB], f32, kind="ExternalInput")
    tri_d = nc.dram_tensor("tri", [P, P], f32, kind="ExternalInput")
    id_d = nc.dram_tensor("ident", [P, P], f32, kind="ExternalInput")
    on_d = nc.dram_tensor("onesin", [1, DH], f32, kind="ExternalInput")
    vo_d = nc.dram_tensor("vones", [P, NKB * HPC], f32, kind="ExternalInput")
    out_d = nc.dram_tensor("out", [N, DIM], f32, kind="ExternalOutput")

    with tile.TileContext(nc) as tc:
        with (
            tc.tile_pool(name="consts", bufs=1) as consts,
            tc.tile_pool(name="wpool", bufs=1) as wpool,
            tc.tile_pool(name="big", bufs=1) as big,
        ):
            # ---- constant / weight loads
            ident = consts.tile([P, P], f32r)
            nc.gpsimd.dma_start(ident[:], id_d[:])
            tri = consts.tile([P, P], f32r)
            nc.gpsimd.dma_start(tri[:], tri_d[:])
            gammat = consts.tile([P, 8], f32)
            nc.sync.dma_start(gammat[:], gm_d[:])
            maskb = consts.tile([P, NKB], f32)
            nc.sync.dma_start(maskb[:], mb_d[:])
            onesr = consts.tile([1, DH], f32r)
            nc.gpsimd.dma_start(onesr[:], on_d[:])

            wq = wpool.tile([P, 8, HPC * DH], f32r)
            wk = wpool.tile([P, 8, HPC * DH], f32r)
            wv = wpool.tile([P, 8, HPC * DH], f32r)
            nc.gpsimd.dma_start(wq[:], wq_d.ap().rearrange("(k p) c -> p k c", p=P))
            nc.gpsimd.dma_start(wk[:], wk_d.ap().rearrange("(k p) c -> p k c", p=P))
            nc.gpsimd.dma_start(wv[:], wv_d.ap().rearrange("(k p) c -> p k c", p=P))
            wo = wpool.tile([P, 2, DIM], f32r)
            nc.gpsimd.dma_start(wo[:], wo_d.ap().rearrange("(hp p) c -> p hp c", p=P))
            # fold gamma into W rows: W[f, :] *= gamma[f]
            for k in range(8):
                for w in (wq, wk, wv):
                    nc.vector.tensor_scalar(
                        out=w[:, k, :], in0=w[:, k, :],
                        scalar1=gammat[:, k:k + 1], scalar2=None, op0=ALU.mult)

            # ---- persistent activations
            qt = big.tile([P, 2, N], f32r)     # Q^T: [d-of-pair, hp, seq]
            kt = big.tile([P, 2, N], f32r)
            vt = big.tile([P, NKB, HPC, DH + 1], f32r)   # V rows + ones col
            nc.gpsimd.dma_start(
                vt[:, :, :, DH:DH + 1],
                vo_d.ap().rearrange("p (kb h) -> p kb h", h=HPC).unsqueeze(3))
            a0 = big.tile([P, N], f32r)        # A^T for head pair 0
            a1 = big.tile([P, N], f32r)
            ss = big.tile([P, 16], f32)        # row sum-of-squares
            sfac = big.tile([P, 16], f32)      # 32 / max(sqrt(ss), 1e-12)

            # ===== merged pipeline: per row-chunk rc, do norm+transpose+
            # projections, then attention for q-chunk qc=rc (all needed
            # K/V blocks kb <= 4*rc+3 are ready). PSUM budget: ps1(2) +
            # S(2x2) + O(1x2) = 8 banks.
            with (
                tc.tile_pool(name="xin", bufs=4) as xin,
                tc.tile_pool(name="sq", bufs=1) as sqp,
                tc.tile_pool(name="xn", bufs=3) as xnp,
                tc.tile_pool(name="xnt", bufs=1) as xntp,
                tc.tile_pool(name="pt", bufs=3) as ptp,
                tc.tile_pool(name="nrm", bufs=2) as nrm,
                tc.tile_pool(name="ps1", bufs=2, space="PSUM") as ps1,
                tc.tile_pool(name="sps", bufs=2, space="PSUM") as sps,
                tc.tile_pool(name="ops", bufs=1, space="PSUM") as ops,
            ):
                for rc in range(RC):
                    xtiles = []
                    for t in range(4):
                        ti = rc * 4 + t
                        xt = xin.tile([P, DIM], f32, tag="x")
                        nc.sync.dma_start(xt[:], x_d[ti * P:(ti + 1) * P, :])
                        xtiles.append(xt)
                        scr = sqp.tile([P, DIM], bf16, tag="sq")
                        nc.scalar.activation(scr[:], xt[:], AF.Square,
                                             accum_out=ss[:, ti:ti + 1])
                    # stats for this row chunk ([128, 4] batch)
                    sl = slice(rc * 4, rc * 4 + 4)
                    nc.scalar.activation(sfac[:, sl], ss[:, sl], AF.Sqrt)
                    nc.vector.tensor_scalar(out=sfac[:, sl], in0=sfac[:, sl],
                                            scalar1=1e-12, scalar2=None,
                                            op0=ALU.max)
                    nc.vector.reciprocal(sfac[:, sl], sfac[:, sl])
                    nc.vector.tensor_scalar(out=sfac[:, sl], in0=sfac[:, sl],
                                            scalar1=float(DIM ** 0.5), scalar2=None,
                                            op0=ALU.mult)

                    xnt = xntp.tile([P, 8, QCHUNK], f32r, tag="xnt")
                    for t in range(4):
                        ti = rc * 4 + t
                        xn = xnp.tile([P, DIM], f32r, tag="xn")
                        nc.vector.tensor_scalar(out=xn[:], in0=xtiles[t][:],
                                                scalar1=sfac[:, ti:ti + 1],
                                                scalar2=None, op0=ALU.mult)
                        for kg in range(2):
                            tp = ps1.tile([P, 4, P], f32r, tag="ps1")
                            for k4 in range(4):
                                k = kg * 4 + k4
                                nc.tensor.transpose(tp[:, k4, :],
                                                    xn[:, k * P:(k + 1) * P],
                                                    ident[:])
                            nc.vector.tensor_copy(
                                xnt[:, kg * 4:(kg + 1) * 4, t * P:(t + 1) * P],
                                tp[:])

                    # Q/K projections for this row chunk
                    for w, dst in ((wq, qt), (wk, kt)):
                        for cc in range(2):
                            ps = ps1.tile([P, QCHUNK], f32, tag="ps1")
                            for k in range(8):
                                nc.tensor.matmul(
                                    ps[:], w[:, k, cc * P:(cc + 1) * P],
                                    xnt[:, k, :],
                                    start=(k == 0), stop=(k == 7))
                            nc.vector.tensor_copy(
                                dst[:, cc, rc * QCHUNK:(rc + 1) * QCHUNK], ps[:])
                    # V projection for the 4 key blocks of this row chunk
                    for t in range(4):
                        kb = rc * 4 + t
                        ps = ps1.tile([P, HPC * DH], f32, tag="ps1")
                        for k in range(8):
                            nc.tensor.matmul(
                                ps[:], xnt[:, k, t * P:(t + 1) * P],
                                wv[:, k, :],
                                start=(k == 0), stop=(k == 7))
                        nc.vector.tensor_copy(
                            vt[:, kb, :, 0:DH], ps[:].rearrange("p (h d) -> p h d", d=DH))

                    # ---- attention for q-chunk qc=rc, both head pairs
                    qc = rc
                    qs = slice(qc * QCHUNK, (qc + 1) * QCHUNK)
                    nkb = 4 * qc + 4
                    for hp, adst in ((0, a0), (1, a1)):
                        ot = ops.tile([DH + 1, 2, QCHUNK], f32, tag="o")
                        for kb in range(nkb):
                            ks = slice(kb * P, (kb + 1) * P)
                            st = sps.tile([P, 2, QCHUNK], f32, tag="s")
                            for h in range(2):
                                nc.tensor.matmul(
                                    st[:, h, :],
                                    kt[h * DH:(h + 1) * DH, hp, ks],
                                    qt[h * DH:(h + 1) * DH, hp, qs],
                                    start=True, stop=True,
                                    tile_position=(h * DH, 0))
                            o = max(0, kb * P - qc * QCHUNK)
                            pt = ptp.tile([P, 2, QCHUNK], f32r, tag="pt")
                            nc.scalar.activation(pt[:, :, o:], st[:, :, o:],
                                                 AF.Exp, scale=SCALE,
                                                 bias=maskb[:, kb:kb + 1])
                            if kb >= 4 * qc:  # diagonal block: triangular mask
                                nc.vector.tensor_tensor(
                                    pt[:, :, o:o + P], pt[:, :, o:o + P],
                                    tri[:, None, :].broadcast_to([P, 2, P]),
                                    ALU.mult)
                            for h in range(2):
                                nc.tensor.matmul(
                                    ot[:, h, o:], vt[:, kb, 2 * hp + h, :],
                                    pt[:, h, o:],
                                    start=(kb == 0), stop=(kb == nkb - 1),
                                    skip_group_check=True)
                        # normalize: A = O[0:64] * (1 / O[64])
                        sums = nrm.tile([1, 2, QCHUNK], f32, tag="sums")
                        nc.vector.tensor_copy(sums[:], ot[DH:DH + 1, :, :])
                        rec = nrm.tile([1, 2, QCHUNK], f32r, tag="rec")
                        with nc.allow_low_precision(reason="f32r softmax recip (1.6e-4)"):
                            nc.vector.reciprocal(rec[:], sums[:])
                        bt = sps.tile([DH, 2, QCHUNK], f32, tag="s")
                        for h in range(2):
                            nc.tensor.matmul(bt[:, h, :], onesr[:],
                                             rec[0:1, h, :],
                                             start=True, stop=True)
                        osb = nrm.tile([DH, 2, QCHUNK], f32, tag="osb")
                        nc.scalar.activation(osb[:], ot[0:DH, :, :], AF.Copy)
                        nc.vector.tensor_tensor(adst[0:DH, qs], osb[:, 0, :],
                                                bt[:, 0, :], ALU.mult)
                        ashq = nrm.tile([DH, QCHUNK], f32r, tag="ashq")
                        nc.vector.tensor_tensor(ashq[:], osb[:, 1, :],
                                                bt[:, 1, :], ALU.mult)
                        nc.sync.dma_start(adst[DH:2 * DH, qs], ashq[:])


                # ---- output projection
                with tc.tile_pool(name="outp", bufs=2) as outp:
                    for r in range(N // P):
                        rs = slice(r * P, (r + 1) * P)
                        orow = outp.tile([P, DIM], f32, tag="orow")
                        for cc in range(2):
                            ps = ps1.tile([P, QCHUNK], f32, tag="ps1")
                            for hp, a in ((0, a0), (1, a1)):
                                nc.tensor.matmul(
                                    ps[:], a[:, rs], wo[:, hp, cc * QCHUNK:(cc + 1) * QCHUNK],
                                    start=(hp == 0), stop=(hp == 1))
                            nc.scalar.activation(orow[:, cc * QCHUNK:(cc + 1) * QCHUNK],
                                                 ps[:], AF.Copy)
                        nc.sync.dma_start(out_d[rs, :], orow[:])

    nc.compile()
    return nc


_CACHE = {}


def _get_nc():
    if "nc" not in _CACHE:
        _CACHE["nc"] = _build()
    return _CACHE["nc"]


def kernel(x, mask, gamma, Wq, Wkv, Wo):
    from concourse import bass_utils

    x = np.ascontiguousarray(np.asarray(x, dtype=np.float32))
    mask = np.asarray(mask)
    gamma = np.asarray(gamma, dtype=np.float32)
    Wq = np.asarray(Wq, dtype=np.float32)
    Wkv = np.asarray(Wkv, dtype=np.float32)
    Wo = np.asarray(Wo, dtype=np.float32)

    gammat = np.ascontiguousarray(gamma.reshape(8, P).T)
    tri = (np.arange(P)[None, :] >= np.arange(P)[:, None]).astype(np.float32)
    ident = np.eye(P, dtype=np.float32)

    in_maps = []
    for c in range(NCORES):
        b, hg = divmod(c, NGROUPS)
        cs = slice(hg * HPC * DH, (hg + 1) * HPC * DH)
        mb = np.where(mask[b], 0.0, -1e30).astype(np.float32)
        in_maps.append({
            "x": x[b],
            "wq": np.ascontiguousarray(Wq[:, cs]),
            "wk": np.ascontiguousarray(Wkv[:, :DIM][:, cs]),
            "wv": np.ascontiguousarray(Wkv[:, DIM:][:, cs]),
            "wo": np.ascontiguousarray(Wo[cs, :]),
            "gammat": gammat,
            "maskbias": np.ascontiguousarray(mb.reshape(NKB, P).T),
            "tri": tri,
            "ident": ident,
            "onesin": np.ones((1, DH), dtype=np.float32),
            "vones": np.ones((P, NKB * HPC), dtype=np.float32),
        })

    nc = _get_nc()
    _CACHE["last_in_maps"] = in_maps
    res = bass_utils.run_bass_kernel_spmd(nc, in_maps, core_ids=list(range(NCORES)))
    out = np.zeros((B, N, DIM), dtype=np.float32)
    for c in range(NCORES):
        b = c // NGROUPS
        out[b] += res.results[c]["out"]
    return out
